# revision 10
# baseline (speedup 1.0000x reference)
"""Chamfer loss kernel for Trainium2 (Bass/Tile), 8-core data-parallel.

Problem: p, q ~ (64, 2048, 4) fp32. loss = sum over batch/points of
0.5*(min_pq + min_qp) where min_pq[n] = min_m ||p3_n - q3_m||, p3 = p[..., 1:].

Strategy (v4 — fp16 matmuls, big-tile Act drains, DVE folds, GpSimd PAR):
  - Shard batch 64 -> 8 cores x 8 batches (SPMD, same program, per-core data).
  - Host augments features so ONE K=16 matmul produces the full negated
    squared-distance tile directly in PSUM.  Coordinates and the squared
    norms are stored as DOUBLE-FP16 (hi + lo residual) so the matmul runs
    at the fp16 rate (1 cycle/row; fp32 runs at 4) while dsq comes out
    fp32-accurate — the dataset's nearest-neighbor dsq ~1e-4..1e-3 would
    drown in single-fp16's ~1e-3 absolute quantization error:
        p' = [1, 1, ph, ph, pl, pl, psq_h, psq_l]
        q'' = GSCALE * [-qsq_h, -qsq_l, 2qh, 2ql, 2qh, 2ql, -1, -1]
        p'.q'' = -GSCALE*(|p|^2 + |q|^2 - 2 p.q) = -GSCALE*dsq   ("g")
  - Per batch: 16 row-tiles; each row-tile is 4 matmuls of 512 into ONE
    [128, 2048] fp32 PSUM tile (4 banks).  ScalarE drains the whole tile
    with a single fp32->fp16 COPY (2048 wide) into an SBUF stripe; the
    first stripe lands directly in the column accumulator.
    (Engine ALU facts, measured: DVE tensor_tensor fp16 runs 2x_1p
    (2 out/cycle), tensor_reduce always 1x, GpSimd cannot run tensor ops
    under this toolchain, Act engine can only drain/activate.)
  - Row minima: DVE fp16 fold tree (TT max at 2 out/cycle); levels below
    1024 are fused across tile pairs/quads via multi-dim APs to amortize
    per-instruction overhead (the final 1x tensor_reduce covers 4 tiles).
  - Column minima: DVE elementwise-max chain across stripes; GpSimd
    partition_all_reduce collapses the partition axis (no PE transposes,
    no PSUM contention) and a tiny DMA spreads the replicated row back
    across 128 partitions.
  - sqrt is monotonic so it only touches the 2*2048 per-batch minima:
    DVE fused (x*-1 max 0), Act sqrt(x + 1e-16) with row-sum accumulation.
  - Each core writes a [128, 16] partial-sum tile; host sums * 0.5.
"""

import os
import sys

import numpy as np

if "/opt/trn_rl_repo" not in sys.path:
    sys.path.insert(0, "/opt/trn_rl_repo")

import concourse.bass as bass
import concourse.mybir as mybir
from concourse import bass_isa, library_config
from concourse.tile import TileContext

B, N, D4 = 64, 2048, 4
NCORES = 8
BPC = B // NCORES  # batches per core
PT = 128           # partition tile (rows per stripe)
NI = N // PT       # 16 row tiles per batch
MMF = 512          # matmul free dim (one PSUM bank of fp32)
NJ = N // MMF      # 4 matmul col blocks per stripe
K = 16             # augmented contraction dim (double-fp16 hi/lo split)
GSCALE = 512.0     # power-of-2 scale on g = -dsq: keeps tiny minima out of
                   # the fp16 subnormal range (HW flushes subnormals to zero;
                   # dense clouds have dsq_min ~1e-5).  512*dsq_max ~ 41k
                   # stays under fp16 max 65504.

F32 = mybir.dt.float32
F16 = mybir.dt.float16

LAST_EXEC_NS = None
LAST_PROFILE = None

_NC_CACHE = None


def build_bass():
    from concourse import bacc

    nc = bacc.Bacc(None, target_bir_lowering=False, debug=False)

    # p-augmented and q-augmented share partitions 0-4; p/q split on free axis
    pq = nc.declare_dram_parameter("pq", [K, 2, BPC, N], F16, isOutput=False)
    out = nc.declare_dram_parameter("out", [128, 2 * BPC], F32, isOutput=True)
    outc = nc.declare_dram_parameter("outc", [128, BPC * NI], F32, isOutput=True)

    with TileContext(nc) as tc:
        with (
            tc.tile_pool(name="singles", bufs=1) as singles,
            tc.tile_pool(name="stripes", bufs=5) as stripes,
            tc.tile_pool(name="caccs", bufs=2) as caccs,
            tc.tile_pool(name="junks", bufs=2) as junks,
            tc.tile_pool(name="small", bufs=2) as small,
            tc.tile_pool(name="psmm", bufs=2, space="PSUM") as psmm,
        ):
            pq_sb = singles.tile([K, 2, BPC, N], F16)
            for c in range(BPC):
                eng = (nc.sync, nc.gpsimd)[c % 2]
                eng.dma_start(
                    out=pq_sb[:, :, c:c + 1, :],
                    in_=pq[:, :, c:c + 1, :],
                )

            sums = singles.tile([128, 2 * BPC], F32)
            nc.vector.memset(sums[:], 0.0)
            colsp_all = singles.tile([128, BPC, NI], F16)
            bias_eps = singles.tile([128, 1], F32)
            nc.vector.memset(bias_eps[:], 1e-16)

            for b in range(BPC):
                cacc = caccs.tile([128, N], F16, tag="cacc")
                rowmax = small.tile([128, NI], F32, tag="rowmax")

                f1p = None
                f2q = None

                for i in range(NI):
                    ps = psmm.tile([128, N], F32, tag="ps")
                    for j in range(NJ):
                        nc.tensor.matmul(
                            ps[:, j * MMF:(j + 1) * MMF],
                            lhsT=pq_sb[0:K, 0, b, i * PT:(i + 1) * PT],
                            rhs=pq_sb[0:K, 1, b, j * MMF:(j + 1) * MMF],
                            start=True,
                            stop=True,
                        )

                    # fp32 PSUM -> fp16 SBUF drain, one whole-tile Act copy.
                    # i=0 seeds the column accumulator directly.
                    if i == 0:
                        s16 = cacc
                    else:
                        s16 = stripes.tile([128, N], F16, tag="s16")
                    nc.scalar.copy(s16[:], ps[:])

                    # row maxima level 1: 2048 -> 1024, into half of the
                    # pair buffer so level 2 handles two tiles at once
                    if i % 2 == 0:
                        f1p = junks.tile([128, 2, N // 2], F16, tag="f1p")
                    nc.vector.tensor_tensor(
                        f1p[:, i % 2, :], s16[:, 0:N // 2], s16[:, N // 2:N],
                        mybir.AluOpType.max,
                    )

                    # column max accumulate (serial chain on cacc)
                    if i > 0:
                        nc.vector.tensor_tensor(
                            cacc[:], cacc[:], s16[:], mybir.AluOpType.max
                        )

                    # level 2 per tile-pair: 2x[128,1024] -> [128,2,512]
                    if i % 2 == 1:
                        if i % 4 == 1:
                            f2q = junks.tile([128, 4, N // 4], F16, tag="f2q")
                        h = 2 * ((i // 2) % 2)
                        nc.vector.tensor_tensor(
                            f2q[:, h:h + 2, :],
                            f1p[:, :, 0:N // 4], f1p[:, :, N // 4:N // 2],
                            mybir.AluOpType.max,
                        )

                    # levels 3-4 + reduce per tile-quad:
                    # [128,4,512] -> [128,4,256] -> [128,4,128] -> [128,4]
                    if i % 4 == 3:
                        f3q = junks.tile([128, 4, N // 8], F16, tag="f3q")
                        nc.vector.tensor_tensor(
                            f3q[:], f2q[:, :, 0:N // 8], f2q[:, :, N // 8:N // 4],
                            mybir.AluOpType.max,
                        )
                        f4q = junks.tile([128, 4, N // 16], F16, tag="f4q")
                        nc.vector.tensor_tensor(
                            f4q[:], f3q[:, :, 0:N // 16], f3q[:, :, N // 16:N // 8],
                            mybir.AluOpType.max,
                        )
                        nc.vector.tensor_reduce(
                            rowmax[:, i - 3:i + 1], f4q[:],
                            axis=mybir.AxisListType.X, op=mybir.AluOpType.max,
                        )

                # ---- batch cleanup ----
                # row-path finals
                dsqr = small.tile([128, NI], F32, tag="dsqr")
                nc.vector.tensor_scalar(
                    dsqr[:], rowmax[:], -1.0 / GSCALE, 0.0,
                    mybir.AluOpType.mult, mybir.AluOpType.max,
                )
                distr = small.tile([128, NI], F32, tag="distr")
                nc.scalar.activation(
                    distr[:], dsqr[:], mybir.ActivationFunctionType.Sqrt,
                    bias=bias_eps[:],
                    accum_out=sums[:, 2 * b:2 * b + 1],
                )

                # column minima: collapse partitions on GpSimd (~10us on HW,
                # overlapped with the next batch), spread the replicated row
                # back over 128 partitions via DMA.  The finals are deferred
                # to the end of the program so no engine stalls on the PAR.
                colrep = caccs.tile([128, N], F16, tag="colrep")
                nc.gpsimd.partition_all_reduce(
                    colrep[:], cacc[:], 128, bass_isa.ReduceOp.max
                )
                nc.sync.dma_start(
                    out=colsp_all[:, b, :], in_=colrep[0:1, :]
                )

            # deferred column-path finals, all batches in two ops; the
            # [128, 128] distance tile goes straight to DRAM (host sums it)
            dsqc = singles.tile([128, BPC * NI], F32)
            nc.vector.tensor_scalar(
                dsqc[:], colsp_all[:, :, :], -1.0 / GSCALE, 0.0,
                mybir.AluOpType.mult, mybir.AluOpType.max,
            )
            distc = singles.tile([128, BPC * NI], F32)
            nc.scalar.activation(
                distc[:], dsqc[:], mybir.ActivationFunctionType.Sqrt,
                bias=bias_eps[:],
            )
            nc.gpsimd.dma_start(out=outc[:, :], in_=distc[:])

            nc.sync.dma_start(out=out[:, :], in_=sums[:])

    nc.finalize()
    return nc


def _get_nc():
    global _NC_CACHE
    if _NC_CACHE is None:
        _NC_CACHE = build_bass()
    return _NC_CACHE


def prep_inputs(p, q):
    """Host-side augmentation + per-core sharding (fp16)."""
    p = np.asarray(p, dtype=np.float32)
    q = np.asarray(q, dtype=np.float32)
    # double-fp16 split: x = hi + lo with hi = fp16(x); products of the
    # lo parts are the only dropped term (~2^-22 relative)
    p3 = p[..., 1:].astype(np.float64)  # (B, N, 3)
    q3 = q[..., 1:].astype(np.float64)
    ph = p3.astype(np.float16).astype(np.float64)
    pl = p3 - ph
    qh = q3.astype(np.float16).astype(np.float64)
    ql = q3 - qh
    psq = np.sum(p3 * p3, axis=-1)  # (B, N)
    qsq = np.sum(q3 * q3, axis=-1)
    psq_h = psq.astype(np.float16).astype(np.float64)
    psq_l = psq - psq_h
    qsq_h = qsq.astype(np.float16).astype(np.float64)
    qsq_l = qsq - qsq_h

    phT = np.transpose(ph, (0, 2, 1))
    plT = np.transpose(pl, (0, 2, 1))
    qhT = np.transpose(qh, (0, 2, 1))
    qlT = np.transpose(ql, (0, 2, 1))

    pqT = np.empty((B, K, 2, N), dtype=np.float16)
    pqT[:, 0, 0, :] = 1.0
    pqT[:, 1, 0, :] = 1.0
    pqT[:, 2:5, 0, :] = phT
    pqT[:, 5:8, 0, :] = phT
    pqT[:, 8:11, 0, :] = plT
    pqT[:, 11:14, 0, :] = plT
    pqT[:, 14, 0, :] = psq_h
    pqT[:, 15, 0, :] = psq_l
    pqT[:, 0, 1, :] = -qsq_h * GSCALE
    pqT[:, 1, 1, :] = -qsq_l * GSCALE
    pqT[:, 2:5, 1, :] = (2.0 * GSCALE) * qhT
    pqT[:, 5:8, 1, :] = (2.0 * GSCALE) * qlT
    pqT[:, 8:11, 1, :] = (2.0 * GSCALE) * qhT
    pqT[:, 11:14, 1, :] = (2.0 * GSCALE) * qlT
    pqT[:, 14, 1, :] = -GSCALE
    pqT[:, 15, 1, :] = -GSCALE

    in_maps = []
    for c in range(NCORES):
        sl = slice(c * BPC, (c + 1) * BPC)
        # (b, k, s, n) -> (k, s, b, n)
        in_maps.append({
            "pq": np.ascontiguousarray(np.transpose(pqT[sl], (1, 2, 0, 3))),
        })
    return in_maps


def _install_ntff_shim():
    """The agent image's antenv lacks axon_hooks; recreate it so
    run_bass_kernel_spmd(trace=True) can capture NTFF profiles."""
    import types

    if "antenv.axon_hooks" in sys.modules:
        return
    mod = types.ModuleType("antenv.axon_hooks")
    holder = [None]
    mod.set_axon_ntff_profile_hook = lambda h: holder.__setitem__(0, h)
    mod.get_axon_ntff_profile_hook = lambda: holder[0]
    sys.modules["antenv.axon_hooks"] = mod
    try:
        if "/root/.axon_site/trn_agent_boot" not in sys.path:
            sys.path.insert(0, "/root/.axon_site/trn_agent_boot")
        from trn_boot import _ntff_profile_via_ctypes

        hook = _ntff_profile_via_ctypes("/opt/axon/libaxon_pjrt.so")
        mod.set_axon_ntff_profile_hook(hook)
    except Exception as e:  # degrade to no-trace
        print("ntff shim install failed:", e, file=sys.stderr)


def _best_effort_device_reset():
    """Clear any wedged NRT state left by a previous failed run."""
    try:
        import ctypes

        import jax

        jax.devices()
        lib = ctypes.CDLL("/opt/axon/libaxon_pjrt.so")
        if hasattr(lib, "axon_reset"):
            lib.axon_reset()
    except Exception:
        pass


def kernel(p, q):
    global LAST_EXEC_NS, LAST_PROFILE
    from concourse.bass_utils import run_bass_kernel_spmd

    # pull inputs to host BEFORE any device reset (they may be live jax arrays)
    in_maps = prep_inputs(p, q)
    _best_effort_device_reset()
    nc = _get_nc()
    trace = os.environ.get("CHAMFER_TRACE", "0") == "1"
    if trace:
        _install_ntff_shim()
    res = run_bass_kernel_spmd(nc, in_maps, list(range(NCORES)), trace=trace)
    LAST_EXEC_NS = res.exec_time_ns
    LAST_PROFILE = res.profile_json
    total = 0.0
    for c in range(NCORES):
        total += float(np.asarray(res.results[c]["out"], dtype=np.float64).sum())
        total += float(np.asarray(res.results[c]["outc"], dtype=np.float64).sum())
    return np.float32(0.5 * total)


# revision 11
# speedup vs baseline: 1.0116x; 1.0116x over previous
"""Chamfer loss kernel for Trainium2 (Bass/Tile), 8-core data-parallel.

Problem: p, q ~ (64, 2048, 4) fp32. loss = sum over batch/points of
0.5*(min_pq + min_qp) where min_pq[n] = min_m ||p3_n - q3_m||, p3 = p[..., 1:].

Strategy (v4 — fp16 matmuls, big-tile Act drains, DVE folds, GpSimd PAR):
  - Shard batch 64 -> 8 cores x 8 batches (SPMD, same program, per-core data).
  - Host augments features so ONE K=16 matmul produces the full negated
    squared-distance tile directly in PSUM.  Coordinates and the squared
    norms are stored as DOUBLE-FP16 (hi + lo residual) so the matmul runs
    at the fp16 rate (1 cycle/row; fp32 runs at 4) while dsq comes out
    fp32-accurate — the dataset's nearest-neighbor dsq ~1e-4..1e-3 would
    drown in single-fp16's ~1e-3 absolute quantization error:
        p' = [1, 1, ph, ph, pl, pl, psq_h, psq_l]
        q'' = GSCALE * [-qsq_h, -qsq_l, 2qh, 2ql, 2qh, 2ql, -1, -1]
        p'.q'' = -GSCALE*(|p|^2 + |q|^2 - 2 p.q) = -GSCALE*dsq   ("g")
  - Per batch: 16 row-tiles; each row-tile is 4 matmuls of 512 into ONE
    [128, 2048] fp32 PSUM tile (4 banks).  ScalarE drains the whole tile
    with a single fp32->fp16 COPY (2048 wide) into an SBUF stripe; the
    first stripe lands directly in the column accumulator.
    (Engine ALU facts, measured: DVE tensor_tensor fp16 runs 2x_1p
    (2 out/cycle), tensor_reduce always 1x, GpSimd cannot run tensor ops
    under this toolchain, Act engine can only drain/activate.)
  - Row minima: DVE fp16 fold tree (TT max at 2 out/cycle); levels below
    1024 are fused across tile pairs/quads via multi-dim APs to amortize
    per-instruction overhead (the final 1x tensor_reduce covers 4 tiles).
  - Column minima: DVE elementwise-max chain across stripes; GpSimd
    partition_all_reduce collapses the partition axis (no PE transposes,
    no PSUM contention) and a tiny DMA spreads the replicated row back
    across 128 partitions.
  - sqrt is monotonic so it only touches the 2*2048 per-batch minima:
    DVE fused (x*-1 max 0), Act sqrt(x + 1e-16) with row-sum accumulation.
  - Each core writes a [128, 16] partial-sum tile; host sums * 0.5.
"""

import os
import sys

import numpy as np

if "/opt/trn_rl_repo" not in sys.path:
    sys.path.insert(0, "/opt/trn_rl_repo")

import concourse.bass as bass
import concourse.mybir as mybir
from concourse import bass_isa, library_config
from concourse.tile import TileContext

B, N, D4 = 64, 2048, 4
NCORES = 8
BPC = B // NCORES  # batches per core
PT = 128           # partition tile (rows per stripe)
NI = N // PT       # 16 row tiles per batch
MMF = 512          # matmul free dim (one PSUM bank of fp32)
NJ = N // MMF      # 4 matmul col blocks per stripe
K = 16             # augmented contraction dim (double-fp16 hi/lo split)
GSCALE = 512.0     # power-of-2 scale on g = -dsq: keeps tiny minima out of
                   # the fp16 subnormal range (HW flushes subnormals to zero;
                   # dense clouds have dsq_min ~1e-5).  512*dsq_max ~ 41k
                   # stays under fp16 max 65504.

F32 = mybir.dt.float32
F16 = mybir.dt.float16

LAST_EXEC_NS = None
LAST_PROFILE = None

_NC_CACHE = None


def build_bass():
    from concourse import bacc

    nc = bacc.Bacc(None, target_bir_lowering=False, debug=False)

    # p-augmented and q-augmented share partitions 0-4; p/q split on free axis
    pq = nc.declare_dram_parameter("pq", [K, 2, BPC, N], F16, isOutput=False)
    out = nc.declare_dram_parameter("out", [128, 2 * BPC], F32, isOutput=True)
    outc = nc.declare_dram_parameter("outc", [128, BPC * NI], F32, isOutput=True)

    with TileContext(nc) as tc:
        with (
            tc.tile_pool(name="singles", bufs=1) as singles,
            tc.tile_pool(name="stripes", bufs=5) as stripes,
            tc.tile_pool(name="caccs", bufs=2) as caccs,
            tc.tile_pool(name="junks", bufs=2) as junks,
            tc.tile_pool(name="small", bufs=2) as small,
            tc.tile_pool(name="psmm", bufs=2, space="PSUM") as psmm,
        ):
            pq_sb = singles.tile([K, 2, BPC, N], F16)
            nc.sync.dma_start(out=pq_sb[:, 0, 0:1, :], in_=pq[:, 0, 0:1, :])
            nc.gpsimd.dma_start(out=pq_sb[:, 1, 0:1, :], in_=pq[:, 1, 0:1, :])
            for c in range(1, BPC):
                eng = (nc.sync, nc.gpsimd)[c % 2]
                eng.dma_start(
                    out=pq_sb[:, :, c:c + 1, :],
                    in_=pq[:, :, c:c + 1, :],
                )

            sums = singles.tile([128, 2 * BPC], F32)
            nc.vector.memset(sums[:], 0.0)
            colsp_all = singles.tile([128, BPC, NI], F16)
            bias_eps = singles.tile([128, 1], F32)
            nc.vector.memset(bias_eps[:], 1e-16)
            bias0 = singles.tile([128, 1], F32)
            nc.vector.memset(bias0[:], 0.0)

            for b in range(BPC):
                cacc = caccs.tile([128, N], F16, tag="cacc")
                rowmax = small.tile([128, NI], F32, tag="rowmax")

                f1p = None
                f2q = None

                for i in range(NI):
                    ps = psmm.tile([128, N], F32, tag="ps")
                    for j in range(NJ):
                        nc.tensor.matmul(
                            ps[:, j * MMF:(j + 1) * MMF],
                            lhsT=pq_sb[0:K, 0, b, i * PT:(i + 1) * PT],
                            rhs=pq_sb[0:K, 1, b, j * MMF:(j + 1) * MMF],
                            start=True,
                            stop=True,
                        )

                    # fp32 PSUM -> fp16 SBUF drain, one whole-tile Act copy.
                    # i=0 seeds the column accumulator directly.
                    if i == 0:
                        s16 = cacc
                    else:
                        s16 = stripes.tile([128, N], F16, tag="s16")
                    nc.scalar.copy(s16[:], ps[:])

                    # row maxima level 1: 2048 -> 1024, into half of the
                    # pair buffer so level 2 handles two tiles at once
                    if i % 2 == 0:
                        f1p = junks.tile([128, 2, N // 2], F16, tag="f1p")
                    nc.vector.tensor_tensor(
                        f1p[:, i % 2, :], s16[:, 0:N // 2], s16[:, N // 2:N],
                        mybir.AluOpType.max,
                    )

                    # column max accumulate (serial chain on cacc)
                    if i > 0:
                        nc.vector.tensor_tensor(
                            cacc[:], cacc[:], s16[:], mybir.AluOpType.max
                        )

                    # level 2 per tile-pair: 2x[128,1024] -> [128,2,512]
                    if i % 2 == 1:
                        if i % 4 == 1:
                            f2q = junks.tile([128, 4, N // 4], F16, tag="f2q")
                        h = 2 * ((i // 2) % 2)
                        nc.vector.tensor_tensor(
                            f2q[:, h:h + 2, :],
                            f1p[:, :, 0:N // 4], f1p[:, :, N // 4:N // 2],
                            mybir.AluOpType.max,
                        )

                    # levels 3-4 + reduce per tile-quad:
                    # [128,4,512] -> [128,4,256] -> [128,4,128] -> [128,4]
                    if i % 4 == 3:
                        f3q = junks.tile([128, 4, N // 8], F16, tag="f3q")
                        nc.vector.tensor_tensor(
                            f3q[:], f2q[:, :, 0:N // 8], f2q[:, :, N // 8:N // 4],
                            mybir.AluOpType.max,
                        )
                        f4q = junks.tile([128, 4, N // 16], F16, tag="f4q")
                        nc.vector.tensor_tensor(
                            f4q[:], f3q[:, :, 0:N // 16], f3q[:, :, N // 16:N // 8],
                            mybir.AluOpType.max,
                        )
                        nc.vector.tensor_reduce(
                            rowmax[:, i - 3:i + 1], f4q[:],
                            axis=mybir.AxisListType.X, op=mybir.AluOpType.max,
                        )

                # ---- batch cleanup ----
                # row-path finals (Relu on Act: DVE tensor_scalar has a
                # ~2.2us fixed cost on HW)
                dsqr = small.tile([128, NI], F32, tag="dsqr")
                nc.scalar.activation(
                    dsqr[:], rowmax[:], mybir.ActivationFunctionType.Relu,
                    bias=bias0[:], scale=-1.0 / GSCALE,
                )
                distr = small.tile([128, NI], F32, tag="distr")
                nc.scalar.activation(
                    distr[:], dsqr[:], mybir.ActivationFunctionType.Sqrt,
                    bias=bias_eps[:],
                    accum_out=sums[:, 2 * b:2 * b + 1],
                )

                # column minima: collapse partitions on GpSimd (~10us on HW,
                # overlapped with the next batch), spread the replicated row
                # back over 128 partitions via DMA.  The finals are deferred
                # to the end of the program so no engine stalls on the PAR.
                colrep = caccs.tile([128, N], F16, tag="colrep")
                nc.gpsimd.partition_all_reduce(
                    colrep[:], cacc[:], 128, bass_isa.ReduceOp.max
                )
                nc.sync.dma_start(
                    out=colsp_all[:, b, :], in_=colrep[0:1, :]
                )

            # deferred column-path finals, all batches in two ops; the
            # [128, 128] distance tile goes straight to DRAM (host sums it)
            dsqc = singles.tile([128, BPC * NI], F32)
            nc.scalar.activation(
                dsqc[:], colsp_all[:, :, :], mybir.ActivationFunctionType.Relu,
                bias=bias0[:], scale=-1.0 / GSCALE,
            )
            distc = singles.tile([128, BPC * NI], F32)
            nc.scalar.activation(
                distc[:], dsqc[:], mybir.ActivationFunctionType.Sqrt,
                bias=bias_eps[:],
            )
            nc.gpsimd.dma_start(out=outc[:, :], in_=distc[:])

            nc.sync.dma_start(out=out[:, :], in_=sums[:])

    nc.finalize()
    return nc


def _get_nc():
    global _NC_CACHE
    if _NC_CACHE is None:
        _NC_CACHE = build_bass()
    return _NC_CACHE


def prep_inputs(p, q):
    """Host-side augmentation + per-core sharding (fp16)."""
    p = np.asarray(p, dtype=np.float32)
    q = np.asarray(q, dtype=np.float32)
    # double-fp16 split: x = hi + lo with hi = fp16(x); products of the
    # lo parts are the only dropped term (~2^-22 relative)
    p3 = p[..., 1:].astype(np.float64)  # (B, N, 3)
    q3 = q[..., 1:].astype(np.float64)
    ph = p3.astype(np.float16).astype(np.float64)
    pl = p3 - ph
    qh = q3.astype(np.float16).astype(np.float64)
    ql = q3 - qh
    psq = np.sum(p3 * p3, axis=-1)  # (B, N)
    qsq = np.sum(q3 * q3, axis=-1)
    psq_h = psq.astype(np.float16).astype(np.float64)
    psq_l = psq - psq_h
    qsq_h = qsq.astype(np.float16).astype(np.float64)
    qsq_l = qsq - qsq_h

    phT = np.transpose(ph, (0, 2, 1))
    plT = np.transpose(pl, (0, 2, 1))
    qhT = np.transpose(qh, (0, 2, 1))
    qlT = np.transpose(ql, (0, 2, 1))

    pqT = np.empty((B, K, 2, N), dtype=np.float16)
    pqT[:, 0, 0, :] = 1.0
    pqT[:, 1, 0, :] = 1.0
    pqT[:, 2:5, 0, :] = phT
    pqT[:, 5:8, 0, :] = phT
    pqT[:, 8:11, 0, :] = plT
    pqT[:, 11:14, 0, :] = plT
    pqT[:, 14, 0, :] = psq_h
    pqT[:, 15, 0, :] = psq_l
    pqT[:, 0, 1, :] = -qsq_h * GSCALE
    pqT[:, 1, 1, :] = -qsq_l * GSCALE
    pqT[:, 2:5, 1, :] = (2.0 * GSCALE) * qhT
    pqT[:, 5:8, 1, :] = (2.0 * GSCALE) * qlT
    pqT[:, 8:11, 1, :] = (2.0 * GSCALE) * qhT
    pqT[:, 11:14, 1, :] = (2.0 * GSCALE) * qlT
    pqT[:, 14, 1, :] = -GSCALE
    pqT[:, 15, 1, :] = -GSCALE

    in_maps = []
    for c in range(NCORES):
        sl = slice(c * BPC, (c + 1) * BPC)
        # (b, k, s, n) -> (k, s, b, n)
        in_maps.append({
            "pq": np.ascontiguousarray(np.transpose(pqT[sl], (1, 2, 0, 3))),
        })
    return in_maps


def _install_ntff_shim():
    """The agent image's antenv lacks axon_hooks; recreate it so
    run_bass_kernel_spmd(trace=True) can capture NTFF profiles."""
    import types

    if "antenv.axon_hooks" in sys.modules:
        return
    mod = types.ModuleType("antenv.axon_hooks")
    holder = [None]
    mod.set_axon_ntff_profile_hook = lambda h: holder.__setitem__(0, h)
    mod.get_axon_ntff_profile_hook = lambda: holder[0]
    sys.modules["antenv.axon_hooks"] = mod
    try:
        if "/root/.axon_site/trn_agent_boot" not in sys.path:
            sys.path.insert(0, "/root/.axon_site/trn_agent_boot")
        from trn_boot import _ntff_profile_via_ctypes

        hook = _ntff_profile_via_ctypes("/opt/axon/libaxon_pjrt.so")
        mod.set_axon_ntff_profile_hook(hook)
    except Exception as e:  # degrade to no-trace
        print("ntff shim install failed:", e, file=sys.stderr)


def _best_effort_device_reset():
    """Clear any wedged NRT state left by a previous failed run."""
    try:
        import ctypes

        import jax

        jax.devices()
        lib = ctypes.CDLL("/opt/axon/libaxon_pjrt.so")
        if hasattr(lib, "axon_reset"):
            lib.axon_reset()
    except Exception:
        pass


def kernel(p, q):
    global LAST_EXEC_NS, LAST_PROFILE
    from concourse.bass_utils import run_bass_kernel_spmd

    # pull inputs to host BEFORE any device reset (they may be live jax arrays)
    in_maps = prep_inputs(p, q)
    _best_effort_device_reset()
    nc = _get_nc()
    trace = os.environ.get("CHAMFER_TRACE", "0") == "1"
    if trace:
        _install_ntff_shim()
    res = run_bass_kernel_spmd(nc, in_maps, list(range(NCORES)), trace=trace)
    LAST_EXEC_NS = res.exec_time_ns
    LAST_PROFILE = res.profile_json
    total = 0.0
    for c in range(NCORES):
        total += float(np.asarray(res.results[c]["out"], dtype=np.float64).sum())
        total += float(np.asarray(res.results[c]["outc"], dtype=np.float64).sum())
    return np.float32(0.5 * total)
